# revision 30
# baseline (speedup 1.0000x reference)
"""GCMC GraphConv on 8 TRN2 NeuronCores.

out = ci * segment_sum(((feat * cj) @ W)[src], dst)

Aggregate-then-transform refactoring (linearity of @ W):
  out[d] = ci[d] * ( (sum_{e: dst_e=d} (feat*cj)[src_e]) @ W )

Per-edge staged features featE = (feat*cj)[src] are built on the host
(edge/message sharding with replicated weight, per the sharding hint) in
dst-bucketed order, so the device does only:
  - streaming loads of featE chunks (128 edges x 256 feats, bf16)
  - one-hot S chunks via is_equal against an iota row (DVE)
  - per dst-block PSUM accumulation G^T[fin, d] += F_chunk^T(*)S_chunk
    on the TensorEngine (K = 128 edges per chunk)
  - final out_b = (G^T)^T @ W as two K=128 matmuls, ci scale, DMA out.

dst nodes are LPT-balanced onto 8 cores x 49 blocks x 128 slots so every
block pads to the same C_BLK chunks (~13 = ceil(~1660/128)).
"""

import heapq

import numpy as np
import ml_dtypes

from concourse import bacc, bass, mybir, tile
from concourse.bass_utils import run_bass_kernel_spmd

N_SRC = 50000
N_DST = 50000
N_EDGES = 640000
IN_F = 256
OUT_F = 128

N_CORES = 8
NBLK = 49                      # dst blocks per core
NBINS = N_CORES * NBLK         # 392 blocks of 128 dst slots
BF16 = ml_dtypes.bfloat16


def _host_prep(feat, weight, cj, ci, src, dst):
    featc = feat * cj                          # fold cj (f32)
    Wb = np.ascontiguousarray(weight.astype(BF16))

    src = src.astype(np.int64)
    dst = dst.astype(np.int64)

    # --- LPT-balance dst nodes into 392 (core, block) bins of <=128 slots ---
    deg = np.bincount(dst, minlength=N_DST)
    order = np.argsort(-deg, kind="stable")
    heap = [(0, b) for b in range(NBINS)]
    heapq.heapify(heap)
    bin_of = np.empty(N_DST, dtype=np.int32)
    slot_of = np.empty(N_DST, dtype=np.int32)
    bin_cnt = np.zeros(NBINS, dtype=np.int32)
    for d in order:
        load, b = heapq.heappop(heap)
        bin_of[d] = b
        slot_of[d] = bin_cnt[b]
        bin_cnt[b] += 1
        if bin_cnt[b] < 128:
            heapq.heappush(heap, (load + int(deg[d]), b))

    # --- bucket edges by the (core, block) bin of their dst ---
    e_bin = bin_of[dst]
    e_slot = slot_of[dst]
    counts = np.bincount(e_bin, minlength=NBINS)
    C_BLK = max(1, int(-(-counts.max() // 128)))
    cap = C_BLK * 128

    starts = np.zeros(NBINS + 1, dtype=np.int64)
    np.cumsum(counts, out=starts[1:])
    eorder = np.argsort(e_bin, kind="stable")
    ranks = np.empty(N_EDGES, dtype=np.int64)
    ranks[eorder] = np.arange(N_EDGES) - starts[e_bin[eorder]]
    flat_pos = e_bin * cap + ranks            # position in padded edge grid

    dl_pad = np.full(NBINS * cap, 128.0, dtype=BF16)
    dl_pad[flat_pos] = e_slot.astype(BF16)
    src_pad = np.zeros(NBINS * cap, dtype=np.int64)   # pad -> feat row 0, S kills it
    src_pad[flat_pos] = src
    dl_pad = dl_pad.reshape(N_CORES, NBLK * cap)
    src_pad = src_pad.reshape(N_CORES, NBLK * cap)

    featE_maps = []
    dstl_maps = []
    ci_maps = []
    nchunks = NBLK * C_BLK
    for k in range(N_CORES):
        fE = featc[src_pad[k]].astype(BF16)            # [NBLK*cap, 256]
        # partition-major layout: [128, nchunks*256] so each partition's
        # block slice is one contiguous DMA run (chunk-major, then feat)
        fE = fE.reshape(nchunks, 128, IN_F).transpose(1, 0, 2).reshape(
            128, nchunks * IN_F)
        featE_maps.append(np.ascontiguousarray(fE))
        dstl_maps.append(np.ascontiguousarray(
            dl_pad[k].reshape(NBLK * C_BLK, 128).T))    # [128, NBLK*C_BLK]
        cim = np.zeros((128, NBLK), dtype=np.float32)
        ci_maps.append(cim)

    dmask = np.arange(N_DST)
    b_all = bin_of[dmask]
    k_all = b_all // NBLK
    blk_all = b_all % NBLK
    s_all = slot_of[dmask]
    for k in range(N_CORES):
        m = k_all == k
        ci_maps[k][s_all[m], blk_all[m]] = ci[dmask[m], 0]

    iota = np.tile(np.arange(128, dtype=np.float32).astype(BF16), (128, 1))
    inv = (k_all, blk_all * 128 + s_all)     # out_full[d] = out_core[k][blk*128+s]
    return featE_maps, Wb, iota, dstl_maps, ci_maps, C_BLK, inv


def _build_program(C_BLK):
    cap = C_BLK * 128
    nchunks = NBLK * C_BLK
    nc = bacc.Bacc("TRN2", target_bir_lowering=False, debug=False)
    dt = mybir.dt

    fE_d = nc.dram_tensor("featE", [128, nchunks * IN_F], dt.bfloat16, kind="ExternalInput").ap()
    w_d = nc.dram_tensor("w", [IN_F, OUT_F], dt.bfloat16, kind="ExternalInput").ap()
    iota_d = nc.dram_tensor("iota", [128, 128], dt.bfloat16, kind="ExternalInput").ap()
    dstl_d = nc.dram_tensor("dstl", [128, nchunks], dt.bfloat16, kind="ExternalInput").ap()
    ci_d = nc.dram_tensor("ci", [128, NBLK], dt.float32, kind="ExternalInput").ap()
    out_d = nc.dram_tensor("out", [NBLK * 128, OUT_F], dt.bfloat16, kind="ExternalOutput").ap()



    with tile.TileContext(nc) as tc:
        with tc.tile_pool(name="const", bufs=1) as pc, \
             tc.tile_pool(name="fpool", bufs=6) as pf, \
             tc.tile_pool(name="spool", bufs=6) as psl, \
             tc.tile_pool(name="gpool", bufs=4) as pg, \
             tc.tile_pool(name="opool", bufs=4) as po, \
             tc.tile_pool(name="psumG", bufs=3, space="PSUM") as ppg, \
             tc.tile_pool(name="psumO", bufs=2, space="PSUM") as ppo:
            w0 = pc.tile([128, OUT_F], dt.bfloat16, tag="w0")
            w1 = pc.tile([128, OUT_F], dt.bfloat16, tag="w1")
            iota_t = pc.tile([128, 128], dt.bfloat16, tag="iota")
            dstl_t = pc.tile([128, nchunks], dt.bfloat16, tag="dstl")
            ci_t = pc.tile([128, NBLK], dt.float32, tag="ci")
            nc.sync.dma_start(out=w0[:], in_=w_d[0:128, :])
            nc.sync.dma_start(out=w1[:], in_=w_d[128:256, :])
            nc.sync.dma_start(out=iota_t[:], in_=iota_d[:])
            nc.sync.dma_start(out=dstl_t[:], in_=dstl_d[:])
            nc.sync.dma_start(out=ci_t[:], in_=ci_d[:])

            for bg in range(NBLK):                     # 1 block per F-load
                bhis = [bg]
                ft = pf.tile([128, C_BLK * IN_F], dt.bfloat16, tag="ft")
                nc.sync.dma_start(
                    out=ft[:],
                    in_=fE_d[:, bg * C_BLK * IN_F:(bg + 1) * C_BLK * IN_F])
                for bi, b in enumerate(bhis):
                    c0 = b * C_BLK
                    fof = 0
                    st = psl.tile([128, cap], dt.bfloat16, tag="st")
                    nc.vector.tensor_tensor(
                        out=st[:].rearrange("p (c d) -> p c d", d=128),
                        in0=dstl_t[:, c0:c0 + C_BLK, None].to_broadcast([128, C_BLK, 128]),
                        in1=iota_t[:, None, :].to_broadcast([128, C_BLK, 128]),
                        op=mybir.AluOpType.is_equal)

                    glo = ppg.tile([128, 128], dt.float32, tag="glo")
                    ghi = ppg.tile([128, 128], dt.float32, tag="ghi")
                    for c in range(C_BLK):
                        nc.tensor.matmul(
                            out=glo[:],
                            lhsT=ft[:, fof + c * IN_F:fof + c * IN_F + 128],
                            rhs=st[:, c * 128:(c + 1) * 128],
                            start=(c == 0), stop=(c == C_BLK - 1))
                        nc.tensor.matmul(
                            out=ghi[:],
                            lhsT=ft[:, fof + c * IN_F + 128:fof + (c + 1) * IN_F],
                            rhs=st[:, c * 128:(c + 1) * 128],
                            start=(c == 0), stop=(c == C_BLK - 1))
                    gsb = pg.tile([128, 2 * 128], dt.bfloat16, tag="gsb")
                    nc.scalar.activation(gsb[:, 0:128], glo[:],
                                         mybir.ActivationFunctionType.Copy)
                    nc.scalar.activation(gsb[:, 128:256], ghi[:],
                                         mybir.ActivationFunctionType.Copy)

                    ops = ppo.tile([128, OUT_F], dt.float32, tag="ops")
                    nc.tensor.matmul(out=ops[:], lhsT=gsb[:, 0:128], rhs=w0[:],
                                     start=True, stop=False)
                    nc.tensor.matmul(out=ops[:], lhsT=gsb[:, 128:256], rhs=w1[:],
                                     start=False, stop=True)
                    ot = po.tile([128, OUT_F], dt.bfloat16, tag="ot")
                    nc.scalar.activation(ot[:], ops[:],
                                         mybir.ActivationFunctionType.Copy,
                                         scale=ci_t[:, b:b + 1])
                    nc.scalar.dma_start(out=out_d[b * 128:(b + 1) * 128, :], in_=ot[:])

    nc.compile()
    return nc


def _run(feat, weight, cj, ci, src, dst, trace=False):
    feat = np.asarray(feat, dtype=np.float32)
    weight = np.asarray(weight, dtype=np.float32)
    cj = np.asarray(cj, dtype=np.float32)
    ci = np.asarray(ci, dtype=np.float32)
    src = np.asarray(src)
    dst = np.asarray(dst)

    featE_maps, Wb, iota, dstl_maps, ci_maps, C_BLK, inv = _host_prep(
        feat, weight, cj, ci, src, dst)
    nc = _build_program(C_BLK)

    in_maps = [
        {"featE": featE_maps[k], "w": Wb, "iota": iota,
         "dstl": dstl_maps[k], "ci": ci_maps[k]}
        for k in range(N_CORES)
    ]
    res = run_bass_kernel_spmd(nc, in_maps, core_ids=list(range(N_CORES)),
                               trace=trace)
    k_all, pos_all = inv
    outs = [np.asarray(res.results[k]["out"]).astype(np.float32)
            for k in range(N_CORES)]
    out = np.empty((N_DST, OUT_F), dtype=np.float32)
    for k in range(N_CORES):
        m = k_all == k
        out[m] = outs[k][pos_all[m]]
    return out, res.exec_time_ns


def kernel(feat, weight, cj, ci, src, dst):
    out, _ = _run(feat, weight, cj, ci, src, dst)
    return out


# revision 32
# speedup vs baseline: 1.0647x; 1.0647x over previous
"""GCMC GraphConv on 8 TRN2 NeuronCores.

out = ci * segment_sum(((feat * cj) @ W)[src], dst)

Aggregate-then-transform refactoring (linearity of @ W):
  out[d] = ci[d] * ( (sum_{e: dst_e=d} (feat*cj)[src_e]) @ W )

Per-edge staged features featE = (feat*cj)[src] are built on the host
(edge/message sharding with replicated weight, per the sharding hint) in
dst-bucketed order, so the device does only:
  - streaming loads of featE chunks (128 edges x 256 feats, bf16)
  - one-hot S chunks via is_equal against an iota row (DVE)
  - per dst-block PSUM accumulation G^T[fin, d] += F_chunk^T(*)S_chunk
    on the TensorEngine (K = 128 edges per chunk)
  - final out_b = (G^T)^T @ W as two K=128 matmuls, ci scale, DMA out.

dst nodes are LPT-balanced onto 8 cores x 49 blocks x 128 slots so every
block pads to the same C_BLK chunks (~13 = ceil(~1660/128)).
"""

import heapq

import numpy as np
import ml_dtypes

from concourse import bacc, bass, mybir, tile
from concourse.bass_utils import run_bass_kernel_spmd

N_SRC = 50000
N_DST = 50000
N_EDGES = 640000
IN_F = 256
OUT_F = 128

N_CORES = 8
NBLK = 49                      # dst blocks per core
NBINS = N_CORES * NBLK         # 392 blocks of 128 dst slots
BF16 = ml_dtypes.bfloat16


def _host_prep(feat, weight, cj, ci, src, dst):
    featc = feat * cj                          # fold cj (f32)
    Wb = np.ascontiguousarray(weight.astype(BF16))

    src = src.astype(np.int64)
    dst = dst.astype(np.int64)

    # --- LPT-balance dst nodes into 392 (core, block) bins of <=128 slots ---
    deg = np.bincount(dst, minlength=N_DST)
    order = np.argsort(-deg, kind="stable")
    heap = [(0, b) for b in range(NBINS)]
    heapq.heapify(heap)
    bin_of = np.empty(N_DST, dtype=np.int32)
    slot_of = np.empty(N_DST, dtype=np.int32)
    bin_cnt = np.zeros(NBINS, dtype=np.int32)
    for d in order:
        load, b = heapq.heappop(heap)
        bin_of[d] = b
        slot_of[d] = bin_cnt[b]
        bin_cnt[b] += 1
        if bin_cnt[b] < 128:
            heapq.heappush(heap, (load + int(deg[d]), b))

    # --- bucket edges by the (core, block) bin of their dst ---
    e_bin = bin_of[dst]
    e_slot = slot_of[dst]
    counts = np.bincount(e_bin, minlength=NBINS)
    C_BLK = max(1, int(-(-counts.max() // 128)))
    cap = C_BLK * 128

    starts = np.zeros(NBINS + 1, dtype=np.int64)
    np.cumsum(counts, out=starts[1:])
    eorder = np.argsort(e_bin, kind="stable")
    ranks = np.empty(N_EDGES, dtype=np.int64)
    ranks[eorder] = np.arange(N_EDGES) - starts[e_bin[eorder]]
    flat_pos = e_bin * cap + ranks            # position in padded edge grid

    dl_pad = np.full(NBINS * cap, 128.0, dtype=BF16)
    dl_pad[flat_pos] = e_slot.astype(BF16)
    src_pad = np.zeros(NBINS * cap, dtype=np.int64)   # pad -> feat row 0, S kills it
    src_pad[flat_pos] = src
    dl_pad = dl_pad.reshape(N_CORES, NBLK * cap)
    src_pad = src_pad.reshape(N_CORES, NBLK * cap)

    featE_maps = []
    dstl_maps = []
    ci_maps = []
    nchunks = NBLK * C_BLK
    for k in range(N_CORES):
        fE = featc[src_pad[k]].astype(BF16)            # [NBLK*cap, 256]
        # partition-major layout: [128, nchunks*256] so each partition's
        # block slice is one contiguous DMA run (chunk-major, then feat)
        fE = fE.reshape(nchunks, 128, IN_F).transpose(1, 0, 2).reshape(
            128, nchunks * IN_F)
        featE_maps.append(np.ascontiguousarray(fE))
        dstl_maps.append(np.ascontiguousarray(
            dl_pad[k].reshape(NBLK * C_BLK, 128).T))    # [128, NBLK*C_BLK]
        cim = np.zeros((128, NBLK), dtype=np.float32)
        ci_maps.append(cim)

    dmask = np.arange(N_DST)
    b_all = bin_of[dmask]
    k_all = b_all // NBLK
    blk_all = b_all % NBLK
    s_all = slot_of[dmask]
    for k in range(N_CORES):
        m = k_all == k
        ci_maps[k][s_all[m], blk_all[m]] = ci[dmask[m], 0]

    iota = np.tile(np.arange(128, dtype=np.float32).astype(BF16), (128, 1))
    inv = (k_all, blk_all * 128 + s_all)     # out_full[d] = out_core[k][blk*128+s]
    return featE_maps, Wb, iota, dstl_maps, ci_maps, C_BLK, inv


def _build_program(C_BLK):
    cap = C_BLK * 128
    nchunks = NBLK * C_BLK
    nc = bacc.Bacc("TRN2", target_bir_lowering=False, debug=False)
    dt = mybir.dt

    fE_d = nc.dram_tensor("featE", [128, nchunks * IN_F], dt.bfloat16, kind="ExternalInput").ap()
    w_d = nc.dram_tensor("w", [IN_F, OUT_F], dt.bfloat16, kind="ExternalInput").ap()
    iota_d = nc.dram_tensor("iota", [128, 128], dt.bfloat16, kind="ExternalInput").ap()
    dstl_d = nc.dram_tensor("dstl", [128, nchunks], dt.bfloat16, kind="ExternalInput").ap()
    ci_d = nc.dram_tensor("ci", [128, NBLK], dt.float32, kind="ExternalInput").ap()
    out_d = nc.dram_tensor("out", [NBLK * 128, OUT_F], dt.float32, kind="ExternalOutput").ap()



    with tile.TileContext(nc) as tc:
        with tc.tile_pool(name="const", bufs=1) as pc, \
             tc.tile_pool(name="fpool", bufs=6) as pf, \
             tc.tile_pool(name="spool", bufs=6) as psl, \
             tc.tile_pool(name="gpool", bufs=4) as pg, \
             tc.tile_pool(name="opool", bufs=4) as po, \
             tc.tile_pool(name="psumG", bufs=3, space="PSUM") as ppg, \
             tc.tile_pool(name="psumO", bufs=2, space="PSUM") as ppo:
            w0 = pc.tile([128, OUT_F], dt.bfloat16, tag="w0")
            w1 = pc.tile([128, OUT_F], dt.bfloat16, tag="w1")
            iota_t = pc.tile([128, 128], dt.bfloat16, tag="iota")
            dstl_t = pc.tile([128, nchunks], dt.bfloat16, tag="dstl")
            ci_t = pc.tile([128, NBLK], dt.float32, tag="ci")
            nc.sync.dma_start(out=w0[:], in_=w_d[0:128, :])
            nc.sync.dma_start(out=w1[:], in_=w_d[128:256, :])
            nc.sync.dma_start(out=iota_t[:], in_=iota_d[:])
            nc.sync.dma_start(out=dstl_t[:], in_=dstl_d[:])
            nc.sync.dma_start(out=ci_t[:], in_=ci_d[:])

            for bg in range(NBLK):                     # 1 block per F-load
                bhis = [bg]
                ft = pf.tile([128, C_BLK * IN_F], dt.bfloat16, tag="ft")
                nc.sync.dma_start(
                    out=ft[:],
                    in_=fE_d[:, bg * C_BLK * IN_F:(bg + 1) * C_BLK * IN_F])
                for bi, b in enumerate(bhis):
                    c0 = b * C_BLK
                    fof = 0
                    st = psl.tile([128, cap], dt.bfloat16, tag="st")
                    nc.vector.tensor_tensor(
                        out=st[:].rearrange("p (c d) -> p c d", d=128),
                        in0=dstl_t[:, c0:c0 + C_BLK, None].to_broadcast([128, C_BLK, 128]),
                        in1=iota_t[:, None, :].to_broadcast([128, C_BLK, 128]),
                        op=mybir.AluOpType.is_equal)

                    glo = ppg.tile([128, 128], dt.float32, tag="glo")
                    ghi = ppg.tile([128, 128], dt.float32, tag="ghi")
                    for c in range(C_BLK):
                        nc.tensor.matmul(
                            out=glo[:],
                            lhsT=ft[:, fof + c * IN_F:fof + c * IN_F + 128],
                            rhs=st[:, c * 128:(c + 1) * 128],
                            start=(c == 0), stop=(c == C_BLK - 1))
                        nc.tensor.matmul(
                            out=ghi[:],
                            lhsT=ft[:, fof + c * IN_F + 128:fof + (c + 1) * IN_F],
                            rhs=st[:, c * 128:(c + 1) * 128],
                            start=(c == 0), stop=(c == C_BLK - 1))
                    gsb = pg.tile([128, 2 * 128], dt.bfloat16, tag="gsb")
                    nc.scalar.activation(gsb[:, 0:128], glo[:],
                                         mybir.ActivationFunctionType.Copy)
                    nc.scalar.activation(gsb[:, 128:256], ghi[:],
                                         mybir.ActivationFunctionType.Copy)

                    ops = ppo.tile([128, OUT_F], dt.float32, tag="ops")
                    nc.tensor.matmul(out=ops[:], lhsT=gsb[:, 0:128], rhs=w0[:],
                                     start=True, stop=False)
                    nc.tensor.matmul(out=ops[:], lhsT=gsb[:, 128:256], rhs=w1[:],
                                     start=False, stop=True)
                    ot = po.tile([128, OUT_F], dt.float32, tag="ot")
                    nc.scalar.activation(ot[:], ops[:],
                                         mybir.ActivationFunctionType.Copy,
                                         scale=ci_t[:, b:b + 1])
                    nc.scalar.dma_start(out=out_d[b * 128:(b + 1) * 128, :], in_=ot[:])

    nc.compile()
    return nc


def _run(feat, weight, cj, ci, src, dst, trace=False):
    feat = np.asarray(feat, dtype=np.float32)
    weight = np.asarray(weight, dtype=np.float32)
    cj = np.asarray(cj, dtype=np.float32)
    ci = np.asarray(ci, dtype=np.float32)
    src = np.asarray(src)
    dst = np.asarray(dst)

    featE_maps, Wb, iota, dstl_maps, ci_maps, C_BLK, inv = _host_prep(
        feat, weight, cj, ci, src, dst)
    nc = _build_program(C_BLK)

    in_maps = [
        {"featE": featE_maps[k], "w": Wb, "iota": iota,
         "dstl": dstl_maps[k], "ci": ci_maps[k]}
        for k in range(N_CORES)
    ]
    res = run_bass_kernel_spmd(nc, in_maps, core_ids=list(range(N_CORES)),
                               trace=trace)
    k_all, pos_all = inv
    outs = [np.asarray(res.results[k]["out"]).astype(np.float32)
            for k in range(N_CORES)]
    out = np.empty((N_DST, OUT_F), dtype=np.float32)
    for k in range(N_CORES):
        m = k_all == k
        out[m] = outs[k][pos_all[m]]
    return out, res.exec_time_ns


def kernel(feat, weight, cj, ci, src, dst):
    out, _ = _run(feat, weight, cj, ci, src, dst)
    return out
